# revision 1
# baseline (speedup 1.0000x reference)
"""GCNConv Trainium2 kernel: out = segment_sum(features[src], dst) @ W + b.

Strategy (8 NeuronCores, graph partitioned by destination node):
  - Host: partition the 391 dst-node tiles (128 nodes each) across 8 cores
    (LPT balance by edge count).  Edges live with their dst tile.  Features
    are replicated to every core in bf16, split into two 25000-row tables so
    gather indices fit in int16 (dma_gather requirement).
  - Device (per core): dma_gather edge source rows (bf16, 256B rows) in big
    batches; per 128-edge chunk build a one-hot(dst_local) matrix on DVE and
    matmul-accumulate msgs.T @ onehot into PSUM, yielding agg.T per node
    tile; then out.T = W.T @ agg.T on the TensorEngine and a fused
    bias-add on the Scalar engine; DMA out.T tiles to DRAM.
  - Host: transpose + scatter per-core tile outputs back to [50000, 128].
"""

import os
import sys

for _p in ("/opt/trn_rl_repo",):
    if _p not in sys.path and os.path.isdir(_p):
        sys.path.insert(0, _p)

import numpy as np
import ml_dtypes

P = 128
N_NODES = 50000
N_EDGES = 640000
D = 128
NCORES = 8
HALF = 25000          # int16 index-range split of the feature table
NTILE = (N_NODES + P - 1) // P          # 391
NSLOT = (NTILE + NCORES - 1) // NCORES  # 49 node tiles per core
GCHUNK = 24           # chunks (of 128 gathered rows) per dma_gather call
NQUEUES = 4           # SWDGE queues; gather desc-gen contexts run concurrently

BF16 = ml_dtypes.bfloat16


# ---------------------------------------------------------------- host plan

def plan(src, dst):
    """Partition node tiles across cores and lay out padded, chunked edge
    lists.  Chunk counts are shared across cores (max over cores) so the
    single SPMD program fits every core."""
    src = np.asarray(src).astype(np.int64)
    dst = np.asarray(dst).astype(np.int64)
    tile = dst // P
    cnt = np.bincount(tile, minlength=NTILE)

    # LPT assignment of tiles to cores, capacity NSLOT each
    order = np.argsort(-cnt, kind="stable")
    core_tiles = [[] for _ in range(NCORES)]
    load = np.zeros(NCORES)
    for t in order:
        for c in sorted(range(NCORES), key=lambda c: load[c]):
            if len(core_tiles[c]) < NSLOT:
                core_tiles[c].append(int(t))
                load[c] += cnt[t]
                break
    for c in range(NCORES):
        core_tiles[c].sort(key=lambda t: -cnt[t])
        while len(core_tiles[c]) < NSLOT:
            core_tiles[c].append(-1)  # dummy empty tile

    # edges grouped by tile
    edge_order = np.argsort(tile, kind="stable")
    tile_sorted = tile[edge_order]
    starts = np.searchsorted(tile_sorted, np.arange(NTILE))
    ends = np.searchsorted(tile_sorted, np.arange(NTILE), side="right")

    lo_edges = [[None] * NSLOT for _ in range(NCORES)]
    hi_edges = [[None] * NSLOT for _ in range(NCORES)]
    n_lo = np.zeros((NCORES, NSLOT), np.int64)
    n_hi = np.zeros((NCORES, NSLOT), np.int64)
    for c in range(NCORES):
        for s, t in enumerate(core_tiles[c]):
            if t < 0:
                lo_edges[c][s] = hi_edges[c][s] = np.empty(0, np.int64)
                continue
            e = edge_order[starts[t]:ends[t]]
            m = src[e] < HALF
            lo_edges[c][s] = e[m]
            hi_edges[c][s] = e[~m]
            n_lo[c, s] = m.sum()
            n_hi[c, s] = (~m).sum()

    Klo = (-(-n_lo // P)).max(axis=0).astype(int)
    Khi = (-(-n_hi // P)).max(axis=0).astype(int)
    for s in range(NSLOT):  # every slot needs >=1 chunk so PSUM is written
        if Klo[s] + Khi[s] == 0:
            Klo[s] = 1
    KLO, KHI = int(Klo.sum()), int(Khi.sum())

    lo_base = np.concatenate([[0], np.cumsum(Klo)])[:NSLOT]
    hi_base = np.concatenate([[0], np.cumsum(Khi)])[:NSLOT]

    # per-core padded index / dst_local arrays, chunk-major [K, 128]
    idx = np.zeros((NCORES, KLO + KHI, P), np.int16)
    dstl = np.full((NCORES, KLO + KHI, P), -1.0, np.float32)
    for c in range(NCORES):
        for s, t in enumerate(core_tiles[c]):
            base = t * P if t >= 0 else 0
            for K, bases, edges, stream_off, table_off in (
                (Klo[s], lo_base, lo_edges[c][s], 0, 0),
                (Khi[s], hi_base, hi_edges[c][s], KLO, HALF),
            ):
                if K == 0:
                    continue
                e = edges
                b0 = stream_off + bases[s]
                flat_i = idx[c, b0:b0 + K].reshape(-1)
                flat_d = dstl[c, b0:b0 + K].reshape(-1)
                flat_i[: len(e)] = (src[e] - table_off).astype(np.int16)
                flat_d[: len(e)] = (dst[e] - base).astype(np.float32)

    return {
        "core_tiles": core_tiles,
        "Klo": Klo, "Khi": Khi, "KLO": KLO, "KHI": KHI,
        "idx": idx, "dstl": dstl,
    }


def _groups(K):
    """Split stream of K chunks into gather groups of <= GCHUNK chunks."""
    out = []
    c = 0
    while c < K:
        out.append((c, min(c + GCHUNK, K)))
        c = out[-1][1]
    return out


def pack_gidx(idx):
    """[K,128] int16 chunk-major indices -> [128, K*8] dma_gather layout
    (index i of a group at [i%16, i//16], replicated on partitions 16..127)."""
    K = idx.shape[0]
    out = np.zeros((128, K * 8), np.int16)
    for c0, c1 in _groups(K):
        g = idx[c0:c1].reshape(-1)                # i = (c-c0)*128 + lane
        blk = g.reshape(-1, 16).T                 # [16, (c1-c0)*8]
        out[:, c0 * 8:c1 * 8] = np.tile(blk, (8, 1))
    return out


# ---------------------------------------------------------------- program

def build(Klo, Khi, dbg=False):
    import concourse.bass as bass
    import concourse.mybir as mybir
    from concourse import bacc
    import concourse.tile as tile

    KLO, KHI = int(np.sum(Klo)), int(np.sum(Khi))
    NCH = KLO + KHI
    bf16, f32, i16 = mybir.dt.bfloat16, mybir.dt.float32, mybir.dt.int16

    nc = bacc.Bacc("TRN2", debug=dbg, num_swdge_queues=NQUEUES)
    flo = nc.dram_tensor("flo", [HALF, D], bf16, kind="ExternalInput")
    fhi = nc.dram_tensor("fhi", [N_NODES - HALF, D], bf16, kind="ExternalInput")
    gidx = nc.dram_tensor("gidx", [P, NCH * 8], i16, kind="ExternalInput")
    dstl = nc.dram_tensor("dstl", [P, NCH], f32, kind="ExternalInput")
    iota = nc.dram_tensor("iota", [P, P], f32, kind="ExternalInput")
    wmat = nc.dram_tensor("wmat", [P, P], bf16, kind="ExternalInput")
    bcol = nc.dram_tensor("bcol", [P, 1], f32, kind="ExternalInput")
    out = nc.dram_tensor("out", [P, NSLOT * P], f32, kind="ExternalOutput")

    lo_groups, hi_groups = _groups(KLO), _groups(KHI)

    with tile.TileContext(nc) as tc:
        with tc.tile_pool(name="const", bufs=1) as cp, \
             tc.tile_pool(name="gat", bufs=3) as gp, \
             tc.tile_pool(name="oh", bufs=4) as ohp, \
             tc.tile_pool(name="res", bufs=3) as resp, \
             tc.tile_pool(name="psA", bufs=2, space="PSUM") as psA, \
             tc.tile_pool(name="psB", bufs=2, space="PSUM") as psB, \
             tc.tile_pool(name="psC", bufs=1, space="PSUM") as psC:

            iota_sb = cp.tile([P, P], f32)
            nc.sync.dma_start(out=iota_sb[:], in_=iota[:])
            iota_t = psC.tile([P, P], f32, tag="iota")
            nc.scalar.copy(out=iota_t[:], in_=iota_sb[:])
            w_t = cp.tile([P, P], bf16)
            nc.sync.dma_start(out=w_t[:], in_=wmat[:])
            b_t = cp.tile([P, 1], f32)
            nc.sync.dma_start(out=b_t[:], in_=bcol[:])
            gidx_t = cp.tile([P, NCH * 8], i16)
            for c0, c1 in [(S + a, S + b) for S, gs in ((0, lo_groups), (KLO, hi_groups))
                           for a, b in gs]:
                nc.sync.dma_start(out=gidx_t[:, c0 * 8:c1 * 8],
                                  in_=gidx[:, c0 * 8:c1 * 8])
            dstl_t = cp.tile([P, NCH], f32)
            nc.sync.dma_start(out=dstl_t[:], in_=dstl[:])

            # per-stream gather state: (groups, table, chunk cursor)
            st = {
                "lo": {"groups": lo_groups, "tab": flo, "g": 0,
                       "tile": None, "c0": 0, "c1": 0, "coff": 0},
                "hi": {"groups": hi_groups, "tab": fhi, "g": 0,
                       "tile": None, "c0": 0, "c1": 0, "coff": KLO},
            }

            def fetch(S):
                c0, c1 = S["groups"][S["g"]]
                n = c1 - c0
                t = gp.tile([P, n * P], mybir.dt.bfloat16,
                            tag="g" + ("lo" if S is st["lo"] else "hi"))
                nc.gpsimd.dma_gather(
                    out_ap=t[:].rearrange("p (g d) -> p g d", d=P),
                    in_ap=S["tab"][:],
                    idxs_ap=gidx_t[:, (S["coff"] + c0) * 8:(S["coff"] + c1) * 8],
                    num_idxs=n * P,
                    num_idxs_reg=n * P,
                    elem_size=P,
                    single_packet=False,
                )
                S["tile"], S["c0"], S["c1"] = t, c0, c1
                S["g"] += 1

            cur = {"lo": 0, "hi": 0}  # global chunk cursor per stream
            for s in range(NSLOT):
                nch = int(Klo[s] + Khi[s])
                ps_agg = psA.tile([P, P], f32, tag="agg")
                ci = 0
                for name, k in (("lo", int(Klo[s])), ("hi", int(Khi[s]))):
                    S = st[name]
                    for _ in range(k):
                        pos = cur[name]
                        if S["tile"] is None or pos >= S["c1"]:
                            fetch(S)
                        off = pos - S["c0"]
                        oh = ohp.tile([P, P], mybir.dt.bfloat16, tag="oh")
                        dc = S["coff"] + pos
                        nc.vector.tensor_scalar(
                            out=oh[:], in0=iota_t[:],
                            scalar1=dstl_t[:, dc:dc + 1], scalar2=None,
                            op0=mybir.AluOpType.is_equal,
                        )
                        nc.tensor.matmul(
                            out=ps_agg[:],
                            lhsT=S["tile"][:, off * P:(off + 1) * P],
                            rhs=oh[:],
                            start=(ci == 0), stop=(ci == nch - 1),
                        )
                        cur[name] += 1
                        ci += 1

                aggT = resp.tile([P, P], mybir.dt.bfloat16, tag="aggT")
                nc.scalar.copy(out=aggT[:], in_=ps_agg[:])
                ps_out = psB.tile([P, P], f32, tag="out")
                nc.tensor.matmul(out=ps_out[:], lhsT=w_t[:], rhs=aggT[:],
                                 start=True, stop=True)
                o_sb = resp.tile([P, P], f32, tag="osb")
                nc.scalar.activation(
                    out=o_sb[:], in_=ps_out[:],
                    func=mybir.ActivationFunctionType.Identity,
                    bias=b_t[:, 0:1],
                )
                nc.sync.dma_start(out=out[:, s * P:(s + 1) * P], in_=o_sb[:])

    # Spread gathers across SWDGE queues.  Tile assigns each Pool-engine DMA
    # a DMASW completion lane in *scheduled* order; queue choice must be a
    # function of that lane (the sim/ucode bind each lane to one queue), so
    # retag after scheduling: queue = lane % NQUEUES.
    for inst in nc.inst_map.values():
        if isinstance(inst, mybir.InstDMAGatherAnt):
            proc = inst.bass_scheduled_proc
            if proc is not None and 11 <= proc <= 18:
                inst.queue_num = (proc - 11) % NQUEUES

    nc.compile()
    return nc


# ---------------------------------------------------------------- in_maps

def make_in_maps(features, W, b, pl):
    f16 = np.ascontiguousarray(features).astype(BF16)
    iota_np = np.tile(np.arange(P, dtype=np.float32)[None, :], (P, 1))
    w_np = np.asarray(W, np.float32).astype(BF16)
    b_np = np.asarray(b, np.float32).reshape(1, D).T.copy()  # [128,1]
    in_maps = []
    for c in range(NCORES):
        in_maps.append({
            "flo": f16[:HALF],
            "fhi": f16[HALF:],
            "gidx": pack_gidx(pl["idx"][c]),
            "dstl": np.ascontiguousarray(pl["dstl"][c].T),
            "iota": iota_np,
            "wmat": w_np,
            "bcol": b_np,
        })
    return in_maps


def unshard(outs, core_tiles):
    """outs: list of {'out': [128, NSLOT*128] f32} per core -> [50000,128]."""
    full = np.zeros((N_NODES, D), np.float32)
    for c in range(NCORES):
        oT = np.asarray(outs[c]["out"], np.float32)
        for s, t in enumerate(core_tiles[c]):
            if t < 0:
                continue
            n0 = t * P
            n1 = min(n0 + P, N_NODES)
            full[n0:n1, :] = oT[:, s * P:s * P + (n1 - n0)].T
    return full


# ---------------------------------------------------------------- entry

_CACHE = {}


def kernel(features, src, dst, W, b):
    from concourse.bass_utils import run_bass_kernel_spmd

    pl = plan(src, dst)
    key = (tuple(pl["Klo"]), tuple(pl["Khi"]))
    if key not in _CACHE:
        _CACHE[key] = build(pl["Klo"], pl["Khi"])
    nc = _CACHE[key]
    in_maps = make_in_maps(features, W, b, pl)
    last = None
    for _ in range(3):  # retry: a previously wedged pool device can fail a load
        try:
            res = run_bass_kernel_spmd(nc, in_maps, core_ids=list(range(NCORES)))
            return unshard(res.results, pl["core_tiles"])
        except Exception as e:  # noqa: BLE001
            last = e
    raise last

